# revision 3
# baseline (speedup 1.0000x reference)
"""DETR-style Hungarian matching loss on 8 Trainium2 NeuronCores.

Sharding: data-parallel over batch. B=16 samples, 2 per core. Each core
computes, for its samples, the four per-pair matrices that the loss needs:
  cost[q,g] = L1(q,g) - (iou(q,g) - bg_ratio(q,g)) - p[q, gt_label[g]]
  nll[q,g]  = logsumexp_c(log(clip(p[q,:]))) - log(clip(p[q, gt_label[g]]))
  l1m[q,g]  = sum_coord |bbox_pred[q] - bbox_gt[g]|
  ioub[q,g] = iou(q,g) - bg_ratio(q,g)
The Hungarian assignment (inherently sequential, host-side in the reference
too) runs on CPU over the device-computed cost matrices; the final scalar is
assembled from the device matrices by pure gathers.
"""

import sys

if "/opt/trn_rl_repo" not in sys.path:
    sys.path.insert(0, "/opt/trn_rl_repo")

import numpy as np

B, Q, G, C = 16, 100, 100, 100
N_CORES = 8
S = B // N_CORES  # samples per core
P = 100           # partitions used (queries / classes)
EPS = 1e-7        # keras backend epsilon

_CACHE = {}


def _build_module():
    import concourse.bacc as bacc
    import concourse.mybir as mybir
    import concourse.tile as tile
    from concourse.masks import make_identity

    f32 = mybir.dt.float32
    Alu = mybir.AluOpType
    Act = mybir.ActivationFunctionType
    X = mybir.AxisListType.X

    nc = bacc.Bacc("TRN2", target_bir_lowering=False, debug=False,
                   num_devices=N_CORES)

    lp = nc.dram_tensor("lp", [S, Q, C], f32, kind="ExternalInput").ap()
    bp = nc.dram_tensor("bp", [S, Q, 4], f32, kind="ExternalInput").ap()
    bg = nc.dram_tensor("bg", [S, G, 4], f32, kind="ExternalInput").ap()
    oh = nc.dram_tensor("oh", [S, C, G], f32, kind="ExternalInput").ap()

    cost_o = nc.dram_tensor("cost", [S, Q, G], f32, kind="ExternalOutput").ap()
    nll_o = nc.dram_tensor("nll", [S, Q, G], f32, kind="ExternalOutput").ap()
    l1m_o = nc.dram_tensor("l1m", [S, Q, G], f32, kind="ExternalOutput").ap()
    ioub_o = nc.dram_tensor("ioub", [S, Q, G], f32, kind="ExternalOutput").ap()

    with tile.TileContext(nc) as tc:
        with tc.tile_pool(name="sb", bufs=1) as sb, \
             tc.tile_pool(name="cst", bufs=1) as cst, \
             tc.tile_pool(name="ps", bufs=1, space="PSUM") as ps:

            ident = cst.tile([P, P], f32, tag="ident")
            make_identity(nc, ident[:])

            # ---- input DMAs ----
            lp2 = sb.tile([P, S, C], f32, tag="lp2")     # partition = q
            nc.sync.dma_start(lp2[:], lp.rearrange("s q c -> q s c"))
            oh2 = sb.tile([P, S, G], f32, tag="oh2")     # partition = c
            nc.sync.dma_start(oh2[:], oh.rearrange("s c g -> c s g"))
            bp2 = sb.tile([P, S, 4], f32, tag="bp2")     # partition = q
            nc.sync.dma_start(bp2[:], bp.rearrange("s q c -> q s c"))
            # bbox_gt broadcast to all partitions: every partition holds the
            # full [S,G,4] gt array so g-terms can live on the free axis.
            B2 = sb.tile([P, S, G, 4], f32, tag="B2")
            nc.sync.dma_start(B2[:], bg[None].broadcast_to((P, S, G, 4)))

            # ---- gt-derived tiles (redundant per partition, g on free) ----
            halfwh = sb.tile([P, S, G, 2], f32, tag="halfwh")
            nc.vector.tensor_scalar(halfwh[:], B2[:, :, :, 2:4], 0.5, None, Alu.mult)
            gul = sb.tile([P, S, G, 2], f32, tag="gul")
            nc.vector.tensor_tensor(gul[:], B2[:, :, :, 0:2], halfwh[:], Alu.subtract)
            gdr = sb.tile([P, S, G, 2], f32, tag="gdr")
            nc.vector.tensor_tensor(gdr[:], B2[:, :, :, 0:2], halfwh[:], Alu.add)
            gwt = sb.tile([P, S, G, 2], f32, tag="gwt")
            nc.vector.tensor_tensor(gwt[:], gdr[:], gul[:], Alu.subtract)
            gwh1 = sb.tile([P, S, G, 2], f32, tag="gwh1")
            nc.scalar.activation(gwh1[:], gwt[:], Act.Relu, bias=1.0)
            garea = sb.tile([P, S, G], f32, tag="garea")
            nc.vector.tensor_tensor(garea[:], gwh1[:, :, :, 0], gwh1[:, :, :, 1], Alu.mult)

            # ---- pred-derived (tiny, per-query columns) ----
            ph = sb.tile([P, S, 2], f32, tag="ph")
            nc.vector.tensor_scalar(ph[:], bp2[:, :, 2:4], 0.5, None, Alu.mult)
            pul = sb.tile([P, S, 2], f32, tag="pul")
            nc.vector.tensor_tensor(pul[:], bp2[:, :, 0:2], ph[:], Alu.subtract)
            pdr = sb.tile([P, S, 2], f32, tag="pdr")
            nc.vector.tensor_tensor(pdr[:], bp2[:, :, 0:2], ph[:], Alu.add)
            pwt = sb.tile([P, S, 2], f32, tag="pwt")
            nc.vector.tensor_tensor(pwt[:], pdr[:], pul[:], Alu.subtract)
            pwh1 = sb.tile([P, S, 2], f32, tag="pwh1")
            nc.scalar.activation(pwh1[:], pwt[:], Act.Relu, bias=1.0)
            parea = sb.tile([P, S, 1], f32, tag="parea")
            nc.vector.tensor_tensor(parea[:], pwh1[:, :, 0:1], pwh1[:, :, 1:2], Alu.mult)

            pulb = pul[:, :, None, :].broadcast_to((P, S, G, 2))
            pdrb = pdr[:, :, None, :].broadcast_to((P, S, G, 2))

            # ---- intersection ----
            lo = sb.tile([P, S, G, 2], f32, tag="lo")
            nc.vector.tensor_tensor(lo[:], gul[:], pulb, Alu.max)
            hi = sb.tile([P, S, G, 2], f32, tag="hi")
            nc.vector.tensor_tensor(hi[:], gdr[:], pdrb, Alu.min)
            iwt = sb.tile([P, S, G, 2], f32, tag="iwt")
            nc.vector.tensor_tensor(iwt[:], hi[:], lo[:], Alu.subtract)
            iwh = sb.tile([P, S, G, 2], f32, tag="iwh")
            nc.scalar.activation(iwh[:], iwt[:], Act.Relu, bias=1.0)
            inter = sb.tile([P, S, G], f32, tag="inter")
            nc.vector.tensor_tensor(inter[:], iwh[:, :, :, 0], iwh[:, :, :, 1], Alu.mult)

            # ---- bound box ----
            blo = sb.tile([P, S, G, 2], f32, tag="blo")
            nc.vector.tensor_tensor(blo[:], gul[:], pulb, Alu.min)
            bhi = sb.tile([P, S, G, 2], f32, tag="bhi")
            nc.vector.tensor_tensor(bhi[:], gdr[:], pdrb, Alu.max)
            bwt = sb.tile([P, S, G, 2], f32, tag="bwt")
            nc.vector.tensor_tensor(bwt[:], bhi[:], blo[:], Alu.subtract)
            bwh = sb.tile([P, S, G, 2], f32, tag="bwh")
            nc.scalar.activation(bwh[:], bwt[:], Act.Relu, bias=1.0)
            bnd = sb.tile([P, S, G], f32, tag="bnd")
            nc.vector.tensor_tensor(bnd[:], bwh[:, :, :, 0], bwh[:, :, :, 1], Alu.mult)

            # ---- union / iou / bg_ratio ----
            un = sb.tile([P, S, G], f32, tag="un")
            nc.vector.tensor_tensor(un[:], garea[:],
                                    parea[:, :, :].broadcast_to((P, S, G)), Alu.add)
            un2 = sb.tile([P, S, G], f32, tag="un2")
            nc.vector.tensor_tensor(un2[:], un[:], inter[:], Alu.subtract)
            und = sb.tile([P, S, G], f32, tag="und")
            nc.vector.tensor_scalar(und[:], un2[:], 1e-9, None, Alu.max)
            rund = sb.tile([P, S, G], f32, tag="rund")
            nc.vector.reciprocal(rund[:], und[:])
            iou = sb.tile([P, S, G], f32, tag="iou")
            nc.vector.tensor_tensor(iou[:], inter[:], rund[:], Alu.mult)
            bndd = sb.tile([P, S, G], f32, tag="bndd")
            nc.vector.tensor_scalar(bndd[:], bnd[:], 1e-9, None, Alu.max)
            rbnd = sb.tile([P, S, G], f32, tag="rbnd")
            nc.vector.reciprocal(rbnd[:], bndd[:])
            bmu = sb.tile([P, S, G], f32, tag="bmu")
            nc.vector.tensor_tensor(bmu[:], bnd[:], un2[:], Alu.subtract)
            bgr = sb.tile([P, S, G], f32, tag="bgr")
            nc.vector.tensor_tensor(bgr[:], bmu[:], rbnd[:], Alu.mult)
            ioub = sb.tile([P, S, G], f32, tag="ioub")
            nc.vector.tensor_tensor(ioub[:], iou[:], bgr[:], Alu.subtract)
            nc.sync.dma_start(ioub_o.rearrange("s q g -> q s g"), ioub[:])

            # ---- L1 matrix ----
            d4 = sb.tile([P, S, G, 4], f32, tag="d4")
            nc.vector.tensor_tensor(
                d4[:], B2[:], bp2[:, :, None, :].broadcast_to((P, S, G, 4)),
                Alu.subtract)
            a4 = sb.tile([P, S, G, 4], f32, tag="a4")
            nc.scalar.activation(a4[:], d4[:], Act.Abs)
            l1m = sb.tile([P, S, G, 1], f32, tag="l1m")
            nc.vector.reduce_sum(l1m[:], a4[:], axis=X)
            nc.sync.dma_start(l1m_o.rearrange("s q g -> q s g"), l1m[:, :, :, 0])

            # ---- label nll pieces ----
            pc = sb.tile([P, S, C], f32, tag="pc")
            nc.vector.tensor_scalar(pc[:], lp2[:], EPS, 1.0 - EPS, Alu.max, Alu.min)
            s2 = sb.tile([P, S, 1], f32, tag="s2")
            nc.vector.reduce_sum(s2[:], pc[:], axis=X)
            lse = sb.tile([P, S, 1], f32, tag="lse")
            nc.scalar.activation(lse[:], s2[:], Act.Ln)

            # gather p[q, gt_label[g]] = (lp^T)^T @ onehot via PE
            g1c = sb.tile([P, S, G], f32, tag="g1c")
            g1ps = []
            for s in range(S):
                pT_ps = ps.tile([P, P], f32, tag=f"pT{s}")
                nc.tensor.transpose(pT_ps[:], lp2[:, s, :], ident[:])
                pTs = sb.tile([P, P], f32, tag=f"pTs{s}")
                nc.scalar.copy(pTs[:], pT_ps[:])
                g1_ps = ps.tile([P, G], f32, tag=f"g1{s}")
                nc.tensor.matmul(g1_ps[:], pTs[:], oh2[:, s, :], start=True, stop=True)
                g1ps.append(g1_ps)
                nc.vector.tensor_scalar(g1c[:, s, :], g1_ps[:], EPS, 1.0 - EPS,
                                        Alu.max, Alu.min)

            lg = sb.tile([P, S, G], f32, tag="lg")
            nc.scalar.activation(lg[:], g1c[:], Act.Ln)
            nll = sb.tile([P, S, G], f32, tag="nll")
            nc.vector.tensor_tensor(
                nll[:], lse[:, :, :].broadcast_to((P, S, G)), lg[:], Alu.subtract)
            nc.sync.dma_start(nll_o.rearrange("s q g -> q s g"), nll[:])

            # ---- cost = l1 - ioub - g1 ----
            lio = sb.tile([P, S, G], f32, tag="lio")
            nc.vector.tensor_tensor(lio[:], l1m[:, :, :, 0], ioub[:], Alu.subtract)
            cost = sb.tile([P, S, G], f32, tag="cost")
            for s in range(S):
                nc.vector.tensor_tensor(cost[:, s, :], lio[:, s, :], g1ps[s][:],
                                        Alu.subtract)
            nc.sync.dma_start(cost_o.rearrange("s q g -> q s g"), cost[:])

    nc.compile()
    return nc


def _get_module():
    if "nc" not in _CACHE:
        _CACHE["nc"] = _build_module()
    return _CACHE["nc"]


def _hungarian_batch(cost):
    """cost: [B,Q,G] float64 -> col [B,Q] (matched gt column per query)."""
    try:
        from scipy.optimize import linear_sum_assignment
        cols = np.empty((cost.shape[0], cost.shape[1]), dtype=np.int64)
        for b in range(cost.shape[0]):
            r, c = linear_sum_assignment(cost[b])
            cols[b] = c  # r is arange for square matrices
        return cols
    except ImportError:
        return np.stack([_hungarian(c) for c in cost])


def _hungarian(cost):
    # e-maxx / Jonker-Volgenant port, identical to the reference oracle.
    cost = np.asarray(cost, dtype=np.float64)
    n = cost.shape[0]
    u = np.zeros(n + 1)
    v = np.zeros(n + 1)
    p = np.zeros(n + 1, dtype=np.int64)
    way = np.zeros(n + 1, dtype=np.int64)
    for i in range(1, n + 1):
        p[0] = i
        j0 = 0
        minv = np.full(n + 1, np.inf)
        used = np.zeros(n + 1, dtype=bool)
        while True:
            used[j0] = True
            i0 = p[j0]
            cur = cost[i0 - 1] - u[i0] - v[1:]
            free = ~used[1:]
            better = free & (cur < minv[1:])
            minv[1:][better] = cur[better]
            way[1:][better] = j0
            masked = np.where(free, minv[1:], np.inf)
            j1 = int(np.argmin(masked)) + 1
            delta = masked[j1 - 1]
            uidx = np.nonzero(used)[0]
            u[p[uidx]] += delta
            v[uidx] -= delta
            minv[~used] -= delta
            j0 = j1
            if p[j0] == 0:
                break
        while j0 != 0:
            j1 = way[j0]
            p[j0] = p[j1]
            j0 = j1
    ans = np.zeros(n, dtype=np.int64)
    ans[p[1:] - 1] = np.arange(n)
    return ans


def run_device(bbox_pred, labels_pred, bbox_gt, labels_gt, trace=False):
    """Run the device kernel; returns (cost, nll, l1m, ioub) [B,Q,G] + results obj."""
    from concourse.bass_utils import run_bass_kernel_spmd

    nc = _get_module()
    onehot = (labels_gt[:, None, :] == np.arange(C, dtype=np.int32)[None, :, None])
    onehot = np.ascontiguousarray(onehot.astype(np.float32))  # [B, C, G]

    in_maps = []
    for i in range(N_CORES):
        sl = slice(i * S, (i + 1) * S)
        in_maps.append({
            "lp": np.ascontiguousarray(labels_pred[sl]),
            "bp": np.ascontiguousarray(bbox_pred[sl]),
            "bg": np.ascontiguousarray(bbox_gt[sl]),
            "oh": np.ascontiguousarray(onehot[sl]),
        })
    res = run_bass_kernel_spmd(nc, in_maps, core_ids=list(range(N_CORES)),
                               trace=trace)
    cost = np.concatenate([r["cost"] for r in res.results], axis=0)
    nll = np.concatenate([r["nll"] for r in res.results], axis=0)
    l1m = np.concatenate([r["l1m"] for r in res.results], axis=0)
    ioub = np.concatenate([r["ioub"] for r in res.results], axis=0)
    return cost, nll, l1m, ioub, res


def kernel(bbox_pred, labels_pred, bbox_gt, labels_gt):
    cost, nll, l1m, ioub, _ = run_device(
        np.asarray(bbox_pred, dtype=np.float32),
        np.asarray(labels_pred, dtype=np.float32),
        np.asarray(bbox_gt, dtype=np.float32),
        np.asarray(labels_gt, dtype=np.int32),
    )
    col = _hungarian_batch(cost.astype(np.float64))          # [B,Q]
    bidx = np.arange(B)[:, None]
    qidx = np.arange(Q)[None, :]
    label_loss = nll[bidx, qidx, col].mean(axis=1)           # [B]
    reg_loss = l1m[bidx, qidx, col].sum(axis=1) / (Q * 4.0)  # [B]
    giou = ioub[:, np.arange(Q), np.arange(Q)].mean(axis=1)  # [B]
    per_sample = label_loss + 5.0 * reg_loss + 2.0 * giou
    return np.array(per_sample.sum(), dtype=np.float32)
